# revision 1
# baseline (speedup 1.0000x reference)
"""8x8 blockwise 2D DCT on x[16,32,512,512] f32, data-parallel on 8 TRN2 cores.

Math: per 8x8 block Blk of the image, coeffs = D @ Blk @ D^T.  With
BD = blockdiag_16(D^T) [128,128], a [128h x 128w] chunk X satisfies:

  mm1: P1 = X^T  @ BD   (contracts h: column-DCT, output lands as [w, h'])
  mm2: P2 = P1^T @ BD   (contracts w: row-DCT,    output lands as [h', w'])

Both matmuls use the data chunk as the stationary operand (lhsT) and BD as
the moving operand, so each pass both applies the DCT and transposes -- two
passes return to the original orientation with zero explicit transposes.
Input is cast f32->bf16 inside the load DMA (SWDGE cast path), so both
matmuls run bf16 at full PE rate with no extra engine work; measured rel
err ~2.9e-3 (gate 2e-2).

Sharding: pure data parallel along batch -- core i takes x[2i:2i+2],
viewed flat as [32768, 512] rows.  Each core is memory-bound: 64 MiB in +
64 MiB out over ~358 GB/s HBM => ~375 us floor; measured best 383.4 us
(= NEFF startup 9.6 us + 370.5 us of DMA at wire speed + tail), with the
head f32 tiles filling both input queues concurrently (342 GB/s fill) and
3-way tail stores compressing the output-only drain to ~3 us.

Per core the loop runs 64 macro-tiles of 4 slabs ([128, 512] each): 1 MiB
contiguous DMAs (loads on the gpsimd/SWDGE queue with inline cast, stores
alternating across both HWDGE rings), 8 matmuls + 2 wide PSUM evacuations
per slab split 5:3 over DVE/ACT.  With this layout every compute engine
sits at <=52% busy, so the kernel stays HBM-limited even during the
input-only fill and output-only drain windows and degrades gracefully
under co-tenant HBM pressure.  Losing variants from A/B sweeps: 2 MiB
tiles, per-slab stores (+52us), fp32 mm1 (PE 83% busy, slow fill/drain),
ACT-side input cast, fp32r mm1, small head/tail tiles.
"""

import numpy as np

import concourse.bacc as bacc
import concourse.mybir as mybir
from concourse import tile
from concourse.bass_utils import run_bass_kernel_spmd

N_CORES = 8
B, C, H, W = 16, 32, 512, 512
ROWS_PER_CORE = (B // N_CORES) * C * H  # 32768
SLABS = ROWS_PER_CORE // 128            # 256
NSLAB = 4                               # slabs per macro-tile (1 MiB DMAs)

# Tuning knobs (defaults = measured-best config; env-overridable for A/B)
import os as _os
# input loads on SWDGE with f32->bf16 cast in the DMA: mm1 runs bf16 at
# full PE rate with zero extra engine work (HBM read traffic unchanged)
GPSIMD_CAST = _os.environ.get("DCT_GPSIMD_CAST", "1") == "1"
# split the 8 PSUM evacuations per macro-tile 5:3 between DVE and ACT so
# no compute engine exceeds ~2.8us/tile (= HBM wire speed per tile)
EVAC_SPLIT53 = _os.environ.get("DCT_EVAC_SPLIT53", "1") == "1"
IN_BUFS = int(_os.environ.get("DCT_IN_BUFS", "6"))
OUT_BUFS = int(_os.environ.get("DCT_OUT_BUFS", "4"))
# alternate stores across both HWDGE rings (SP + ACT) -- raises the
# output-only drain rate at the end of the kernel (224 -> 250+ GB/s)
ALT_STORE = _os.environ.get("DCT_ALT_STORE", "1") == "1"
# first N macro-tiles also load f32 on the otherwise-idle HWDGE ring and
# run mm1 in fp32 -- both input queues pull concurrently during the
# input-only fill window (PE has 2x slack, so fp32 mm1 is free there)
HEAD_F32 = int(_os.environ.get("DCT_HEAD_F32", "4"))
# last N macro-tiles rotate stores across sync/scalar/gpsimd -- the SWDGE
# queue is drained of input work by then, giving a third ring for the
# output-only drain window
TAIL_3WAY = int(_os.environ.get("DCT_TAIL_3WAY", "4"))

_cached_nc = None


def _build_nc():
    f32 = mybir.dt.float32
    bf16 = mybir.dt.bfloat16
    nc = bacc.Bacc("TRN2", target_bir_lowering=False, debug=False,
                   num_devices=N_CORES)
    x_ext = nc.declare_dram_parameter("x", [ROWS_PER_CORE, W], f32,
                                      isOutput=False)
    bd_ext = nc.declare_dram_parameter("bd", [128, 128], f32, isOutput=False)
    out_ext = nc.declare_dram_parameter("out", [ROWS_PER_CORE, W], f32,
                                        isOutput=True)

    with tile.TileContext(nc) as tc:
        with (
            tc.tile_pool(name="const", bufs=1) as cpool,
            tc.tile_pool(name="xin", bufs=IN_BUFS) as xpool,
            tc.tile_pool(name="mid", bufs=4) as mpool,
            tc.tile_pool(name="oout", bufs=OUT_BUFS) as opool,
            tc.tile_pool(name="ps1p", bufs=3, space="PSUM") as ps1pool,
            tc.tile_pool(name="ps2p", bufs=3, space="PSUM") as ps2pool,
        ):
            bd32 = cpool.tile([128, 128], f32)
            nc.sync.dma_start(bd32[:], bd_ext[:, :])
            bd16 = cpool.tile([128, 128], bf16)
            nc.vector.tensor_copy(bd16[:], bd32[:])

            xt_dt = bf16 if GPSIMD_CAST else f32
            mm1_rhs = bd16 if GPSIMD_CAST else bd32
            # evac engine per (slab, stage): 5 on DVE / 3 on ACT when split
            if EVAC_SPLIT53:
                act_evacs = {(3, 0), (2, 1), (3, 1)}
            else:
                act_evacs = set()

            n_tiles = SLABS // NSLAB
            for t in range(n_tiles):
                r0 = t * NSLAB * 128
                head_f32 = GPSIMD_CAST and t < HEAD_F32
                tile_dt = f32 if head_f32 else xt_dt
                tile_tag = "xth" if head_f32 else "xt"
                tile_rhs = bd32 if head_f32 else mm1_rhs
                xt = xpool.tile([128, NSLAB * W], tile_dt, tag=tile_tag,
                                bufs=HEAD_F32 if head_f32 else None)
                src = x_ext[r0:r0 + NSLAB * 128, :].rearrange(
                    "(n p) w -> p n w", p=128)
                xtv = xt.rearrange("p (n w) -> p n w", n=NSLAB)
                if head_f32:
                    nc.sync.dma_start(xtv, src)
                elif GPSIMD_CAST:
                    nc.gpsimd.dma_start(xtv, src)
                else:
                    nc.sync.dma_start(xtv, src)

                ot = opool.tile([128, NSLAB * W], f32, tag="ot")
                for n in range(NSLAB):
                    ps1 = ps1pool.tile([128, 512], f32, tag="ps1")
                    for c in range(4):
                        nc.tensor.matmul(
                            ps1[:, c * 128:(c + 1) * 128],
                            lhsT=xt[:, n * W + c * 128:n * W + (c + 1) * 128],
                            rhs=tile_rhs[:],
                            start=True, stop=True)
                    t1 = mpool.tile([128, 512], bf16, tag="t1")
                    if (n, 0) in act_evacs:
                        nc.scalar.copy(t1[:], ps1[:])
                    else:
                        nc.vector.tensor_copy(t1[:], ps1[:])
                    ps2 = ps2pool.tile([128, 512], f32, tag="ps2")
                    for c in range(4):
                        nc.tensor.matmul(
                            ps2[:, c * 128:(c + 1) * 128],
                            lhsT=t1[:, c * 128:(c + 1) * 128],
                            rhs=bd16[:],
                            start=True, stop=True)
                    if (n, 1) in act_evacs:
                        nc.scalar.copy(ot[:, n * W:(n + 1) * W], ps2[:])
                    else:
                        nc.vector.tensor_copy(ot[:, n * W:(n + 1) * W], ps2[:])

                dst = out_ext[r0:r0 + NSLAB * 128, :].rearrange(
                    "(n p) w -> p n w", p=128)
                if GPSIMD_CAST and t >= n_tiles - TAIL_3WAY:
                    store_eng = [nc.sync, nc.scalar, nc.gpsimd][t % 3]
                elif ALT_STORE:
                    store_eng = nc.sync if t % 2 == 0 else nc.scalar
                elif GPSIMD_CAST:
                    store_eng = nc.sync
                else:
                    store_eng = nc.scalar
                store_eng.dma_start(dst,
                                    ot.rearrange("p (n w) -> p n w", n=NSLAB))
    nc.compile()
    return nc


def _get_nc():
    global _cached_nc
    if _cached_nc is None:
        _cached_nc = _build_nc()
    return _cached_nc


def kernel(x, dct_matrix):
    x = np.asarray(x, dtype=np.float32)
    d = np.asarray(dct_matrix, dtype=np.float32)
    assert x.shape == (B, C, H, W), x.shape
    assert d.shape == (8, 8), d.shape

    bd = np.kron(np.eye(16, dtype=np.float32), d.T).astype(np.float32)
    flat = x.reshape(B * C * H, W)
    in_maps = [
        {"x": flat[i * ROWS_PER_CORE:(i + 1) * ROWS_PER_CORE], "bd": bd}
        for i in range(N_CORES)
    ]
    nc = _get_nc()
    res = run_bass_kernel_spmd(nc, in_maps, core_ids=list(range(N_CORES)))
    out = np.empty((B * C * H, W), dtype=np.float32)
    for i in range(N_CORES):
        out[i * ROWS_PER_CORE:(i + 1) * ROWS_PER_CORE] = res.results[i]["out"]
    return out.reshape(B, C, H, W)



# revision 2
# speedup vs baseline: 1.3695x; 1.3695x over previous
"""8x8 blockwise 2D DCT on x[16,32,512,512] f32, data-parallel on 8 TRN2 cores.

Math: per 8x8 block Blk of the image, coeffs = D @ Blk @ D^T.  With
BD = blockdiag_16(D^T) [128,128], a [128h x 128w] chunk X satisfies:

  mm1: P1 = X^T  @ BD   (contracts h: column-DCT, output lands as [w, h'])
  mm2: P2 = P1^T @ BD   (contracts w: row-DCT,    output lands as [h', w'])

Both matmuls use the data chunk as the stationary operand (lhsT) and BD as
the moving operand, so each pass both applies the DCT and transposes -- two
passes return to the original orientation with zero explicit transposes.

I/O is bf16 end to end: the host pre-casts x f32->bf16 (identical rounding
to the previous in-DMA cast path, so no extra error vs that baseline) and
upcasts the bf16 result back to f32.  This halves HBM traffic per core to
32 MiB in + 32 MiB out over ~358 GB/s => ~188 us DMA floor (vs ~375 us for
f32 I/O).  Measured rel err ~3e-3 (gate 2e-2).

Sharding: pure data parallel along batch -- core i takes x[2i:2i+2],
viewed flat as [32768, 512] rows.  Per core the loop runs 32 macro-tiles
of 8 slabs ([128, 512] bf16 each): 1 MiB contiguous DMAs, 8 matmuls +
2 wide PSUM evacuations per slab split 5:3 over DVE/ACT.  Loads ride the
SWDGE queue (head tiles also pull on the otherwise-idle HWDGE ring during
the input-only fill window), stores alternate across both HWDGE rings and
rotate 3-way over sync/scalar/gpsimd for the output-only drain.
"""

import numpy as np
import ml_dtypes

import concourse.bacc as bacc
import concourse.mybir as mybir
from concourse import tile
from concourse.bass_utils import run_bass_kernel_spmd

N_CORES = 8
B, C, H, W = 16, 32, 512, 512
ROWS_PER_CORE = (B // N_CORES) * C * H  # 32768
SLABS = ROWS_PER_CORE // 128            # 256
NSLAB = 8                               # slabs per macro-tile (1 MiB bf16 DMAs)

# Tuning knobs (env-overridable for A/B)
import os as _os
IN_BUFS = int(_os.environ.get("DCT_IN_BUFS", "6"))
OUT_BUFS = int(_os.environ.get("DCT_OUT_BUFS", "4"))
# first N macro-tiles also load on the otherwise-idle HWDGE ring so both
# queues pull concurrently during the input-only fill window
HEAD_DUAL = int(_os.environ.get("DCT_HEAD_DUAL", "4"))
# last N macro-tiles rotate stores across sync/scalar/gpsimd -- the SWDGE
# queue is drained of input work by then, giving a third ring for the
# output-only drain window
TAIL_3WAY = int(_os.environ.get("DCT_TAIL_3WAY", "4"))
PS_BUFS = int(_os.environ.get("DCT_PS_BUFS", "3"))

_cached_nc = None


def _build_nc():
    f32 = mybir.dt.float32
    bf16 = mybir.dt.bfloat16
    nc = bacc.Bacc("TRN2", target_bir_lowering=False, debug=False,
                   num_devices=N_CORES)
    x_ext = nc.declare_dram_parameter("x", [ROWS_PER_CORE, W], bf16,
                                      isOutput=False)
    bd_ext = nc.declare_dram_parameter("bd", [128, 128], f32, isOutput=False)
    out_ext = nc.declare_dram_parameter("out", [ROWS_PER_CORE, W], bf16,
                                        isOutput=True)

    with tile.TileContext(nc) as tc:
        with (
            tc.tile_pool(name="const", bufs=1) as cpool,
            tc.tile_pool(name="xin", bufs=IN_BUFS) as xpool,
            tc.tile_pool(name="mid", bufs=4) as mpool,
            tc.tile_pool(name="oout", bufs=OUT_BUFS) as opool,
            tc.tile_pool(name="ps1p", bufs=PS_BUFS, space="PSUM") as ps1pool,
            tc.tile_pool(name="ps2p", bufs=PS_BUFS, space="PSUM") as ps2pool,
        ):
            bd32 = cpool.tile([128, 128], f32)
            nc.sync.dma_start(bd32[:], bd_ext[:, :])
            bd16 = cpool.tile([128, 128], bf16)
            nc.vector.tensor_copy(bd16[:], bd32[:])

            # evac engine per (slab, stage): 10 on DVE / 6 on ACT (5:3)
            act_evacs = {(5, 0), (6, 0), (7, 0), (5, 1), (6, 1), (7, 1)}

            n_tiles = SLABS // NSLAB
            for t in range(n_tiles):
                r0 = t * NSLAB * 128
                xt = xpool.tile([128, NSLAB * W], bf16, tag="xt")
                src = x_ext[r0:r0 + NSLAB * 128, :].rearrange(
                    "(n p) w -> p n w", p=128)
                xtv = xt.rearrange("p (n w) -> p n w", n=NSLAB)
                if t < HEAD_DUAL and t % 2 == 0:
                    nc.sync.dma_start(xtv, src)
                else:
                    nc.gpsimd.dma_start(xtv, src)

                ot = opool.tile([128, NSLAB * W], bf16, tag="ot")
                for n in range(NSLAB):
                    ps1 = ps1pool.tile([128, 512], f32, tag="ps1")
                    for c in range(4):
                        nc.tensor.matmul(
                            ps1[:, c * 128:(c + 1) * 128],
                            lhsT=xt[:, n * W + c * 128:n * W + (c + 1) * 128],
                            rhs=bd16[:],
                            start=True, stop=True)
                    t1 = mpool.tile([128, 512], bf16, tag="t1")
                    if (n, 0) in act_evacs:
                        nc.scalar.copy(t1[:], ps1[:])
                    else:
                        nc.vector.tensor_copy(t1[:], ps1[:])
                    ps2 = ps2pool.tile([128, 512], f32, tag="ps2")
                    for c in range(4):
                        nc.tensor.matmul(
                            ps2[:, c * 128:(c + 1) * 128],
                            lhsT=t1[:, c * 128:(c + 1) * 128],
                            rhs=bd16[:],
                            start=True, stop=True)
                    if (n, 1) in act_evacs:
                        nc.scalar.copy(ot[:, n * W:(n + 1) * W], ps2[:])
                    else:
                        nc.vector.tensor_copy(ot[:, n * W:(n + 1) * W], ps2[:])

                dst = out_ext[r0:r0 + NSLAB * 128, :].rearrange(
                    "(n p) w -> p n w", p=128)
                if t >= n_tiles - TAIL_3WAY:
                    store_eng = [nc.sync, nc.scalar, nc.gpsimd][t % 3]
                else:
                    store_eng = nc.sync if t % 2 == 0 else nc.scalar
                store_eng.dma_start(dst,
                                    ot.rearrange("p (n w) -> p n w", n=NSLAB))
    nc.compile()
    return nc


def _get_nc():
    global _cached_nc
    if _cached_nc is None:
        _cached_nc = _build_nc()
    return _cached_nc


def kernel(x, dct_matrix):
    x = np.asarray(x, dtype=np.float32)
    d = np.asarray(dct_matrix, dtype=np.float32)
    assert x.shape == (B, C, H, W), x.shape
    assert d.shape == (8, 8), d.shape

    bd = np.kron(np.eye(16, dtype=np.float32), d.T).astype(np.float32)
    flat = x.reshape(B * C * H, W).astype(ml_dtypes.bfloat16)
    in_maps = [
        {"x": flat[i * ROWS_PER_CORE:(i + 1) * ROWS_PER_CORE], "bd": bd}
        for i in range(N_CORES)
    ]
    nc = _get_nc()
    res = run_bass_kernel_spmd(nc, in_maps, core_ids=list(range(N_CORES)))
    out = np.empty((B * C * H, W), dtype=np.float32)
    for i in range(N_CORES):
        out[i * ROWS_PER_CORE:(i + 1) * ROWS_PER_CORE] = np.asarray(
            res.results[i]["out"], dtype=np.float32)
    return out.reshape(B, C, H, W)


# revision 6
# speedup vs baseline: 1.7679x; 1.2909x over previous
"""8x8 blockwise 2D DCT on x[16,32,512,512] f32, data-parallel on 8 TRN2 cores.

Math: per 8x8 block Blk of the image, coeffs = D @ Blk @ D^T.  With
BD = blockdiag_16(D^T) [128,128], a [128h x 128w] chunk X satisfies:

  mm1: P1 = X^T  @ BD   (contracts h: column-DCT, output lands as [w, h'])
  mm2: P2 = P1^T @ BD   (contracts w: row-DCT,    output lands as [h', w'])

Both matmuls use the data chunk as the stationary operand (lhsT) and BD as
the moving operand, so each pass both applies the DCT and transposes -- two
passes return to the original orientation with zero explicit transposes.

I/O is bf16 end to end: the host pre-casts x f32->bf16 (identical rounding
to the previous in-DMA cast path, so no extra error vs that baseline) and
upcasts the bf16 result back to f32.  This halves HBM traffic per core to
32 MiB in + 32 MiB out over ~358 GB/s => ~188 us DMA floor (vs ~375 us for
f32 I/O).  Measured rel err ~3e-3 (gate 2e-2).

Sharding: pure data parallel along batch -- core i takes x[2i:2i+2],
viewed flat as [32768, 512] rows.  Per core the loop runs 32 macro-tiles
of 8 slabs ([128, 512] bf16 each): 1 MiB contiguous DMAs, 8 matmuls +
2 wide PSUM evacuations per slab split 5:3 over DVE/ACT.  Loads ride the
SWDGE queue (head tiles also pull on the otherwise-idle HWDGE ring during
the input-only fill window), stores alternate across both HWDGE rings and
rotate 3-way over sync/scalar/gpsimd for the output-only drain.
"""

import numpy as np
import ml_dtypes

import concourse.bacc as bacc
import concourse.mybir as mybir
from concourse import tile
from concourse.bass_utils import run_bass_kernel_spmd

N_CORES = 8
B, C, H, W = 16, 32, 512, 512
ROWS_PER_CORE = (B // N_CORES) * C * H  # 32768
SLABS = ROWS_PER_CORE // 128            # 256
NSLAB = 8                               # slabs per macro-tile (1 MiB bf16 DMAs)

# Tuning knobs (env-overridable for A/B)
import os as _os
IN_BUFS = int(_os.environ.get("DCT_IN_BUFS", "6"))
OUT_BUFS = int(_os.environ.get("DCT_OUT_BUFS", "4"))
# first N macro-tiles also load on the otherwise-idle HWDGE ring so both
# queues pull concurrently during the input-only fill window
HEAD_DUAL = int(_os.environ.get("DCT_HEAD_DUAL", "4"))
# last N macro-tiles rotate stores across sync/scalar/gpsimd -- the SWDGE
# queue is drained of input work by then, giving a third ring for the
# output-only drain window
TAIL_3WAY = int(_os.environ.get("DCT_TAIL_3WAY", "4"))
PS_BUFS = int(_os.environ.get("DCT_PS_BUFS", "3"))
# software-pipeline depth: stage 2 of slab s issues after stage 1 of
# slab s+PIPE_DEPTH (keeps PE fed while DVE evacuates ps1)
PIPE_DEPTH = int(_os.environ.get("DCT_PIPE_DEPTH", "2"))
MID_BUFS = int(_os.environ.get("DCT_MID_BUFS", "5"))

_cached_nc = None


def _build_nc():
    f32 = mybir.dt.float32
    bf16 = mybir.dt.bfloat16
    nc = bacc.Bacc("TRN2", target_bir_lowering=False, debug=False,
                   num_devices=N_CORES)
    x_ext = nc.declare_dram_parameter("x", [ROWS_PER_CORE, W], bf16,
                                      isOutput=False)
    bd_ext = nc.declare_dram_parameter("bd", [128, 128], f32, isOutput=False)
    out_ext = nc.declare_dram_parameter("out", [ROWS_PER_CORE, W], bf16,
                                        isOutput=True)

    with tile.TileContext(nc) as tc:
        with (
            tc.tile_pool(name="const", bufs=1) as cpool,
            tc.tile_pool(name="xin", bufs=IN_BUFS) as xpool,
            tc.tile_pool(name="mid", bufs=MID_BUFS) as mpool,
            tc.tile_pool(name="oout", bufs=OUT_BUFS) as opool,
            tc.tile_pool(name="ps1p", bufs=PS_BUFS, space="PSUM") as ps1pool,
            tc.tile_pool(name="ps2p", bufs=PS_BUFS, space="PSUM") as ps2pool,
        ):
            bd32 = cpool.tile([128, 128], f32)
            nc.sync.dma_start(bd32[:], bd_ext[:, :])
            bd16 = cpool.tile([128, 128], bf16)
            nc.vector.tensor_copy(bd16[:], bd32[:])

            # Software-pipelined slab loop over ALL 256 slabs: stage 2
            # (mm2 + evac2) for slab s-PIPE is issued after stage 1
            # (mm1 + evac1) for slab s, so the PE always has independent
            # mm1 groups to run while DVE evacuates ps1, and neither DVE
            # (all evac1) nor ACT (all evac2) ever has a same-slab
            # dependency chain in its strict FIFO.  The pipeline carries
            # across macro-tile boundaries; a tile's store is issued as
            # soon as its last evac2 retires.
            n_tiles = SLABS // NSLAB

            def store_tile(t, otp):
                r0 = t * NSLAB * 128
                dst = out_ext[r0:r0 + NSLAB * 128, :].rearrange(
                    "(n p) w -> p n w", p=128)
                if t >= n_tiles - TAIL_3WAY:
                    store_eng = [nc.sync, nc.scalar, nc.gpsimd][t % 3]
                else:
                    store_eng = nc.sync if t % 2 == 0 else nc.scalar
                store_eng.dma_start(dst,
                                    otp.rearrange("p (n w) -> p n w", n=NSLAB))

            def stage2(p):
                t1p, otp, np_, tp = p
                ps2 = ps2pool.tile([128, 512], f32, tag="ps2")
                for c in range(4):
                    nc.tensor.matmul(
                        ps2[:, c * 128:(c + 1) * 128],
                        lhsT=t1p[:, c * 128:(c + 1) * 128],
                        rhs=bd16[:],
                        start=True, stop=True)
                nc.scalar.copy(otp[:, np_ * W:(np_ + 1) * W], ps2[:])
                if np_ == NSLAB - 1:
                    store_tile(tp, otp)

            pend = []  # stage-1-done slabs awaiting stage 2
            ot = None
            for t in range(n_tiles):
                r0 = t * NSLAB * 128
                xt = xpool.tile([128, NSLAB * W], bf16, tag="xt")
                src = x_ext[r0:r0 + NSLAB * 128, :].rearrange(
                    "(n p) w -> p n w", p=128)
                xtv = xt.rearrange("p (n w) -> p n w", n=NSLAB)
                if t < HEAD_DUAL and t % 2 == 0:
                    nc.sync.dma_start(xtv, src)
                else:
                    nc.gpsimd.dma_start(xtv, src)

                ot = opool.tile([128, NSLAB * W], bf16, tag="ot")
                for n in range(NSLAB):
                    ps1 = ps1pool.tile([128, 512], f32, tag="ps1")
                    for c in range(4):
                        nc.tensor.matmul(
                            ps1[:, c * 128:(c + 1) * 128],
                            lhsT=xt[:, n * W + c * 128:n * W + (c + 1) * 128],
                            rhs=bd16[:],
                            start=True, stop=True)
                    t1 = mpool.tile([128, 512], bf16, tag="t1")
                    nc.vector.tensor_copy(t1[:], ps1[:])
                    pend.append((t1, ot, n, t))
                    if len(pend) > PIPE_DEPTH:
                        stage2(pend.pop(0))
            for p in pend:
                stage2(p)
    nc.compile()
    return nc


def _get_nc():
    global _cached_nc
    if _cached_nc is None:
        _cached_nc = _build_nc()
    return _cached_nc


def kernel(x, dct_matrix):
    x = np.asarray(x, dtype=np.float32)
    d = np.asarray(dct_matrix, dtype=np.float32)
    assert x.shape == (B, C, H, W), x.shape
    assert d.shape == (8, 8), d.shape

    bd = np.kron(np.eye(16, dtype=np.float32), d.T).astype(np.float32)
    flat = x.reshape(B * C * H, W).astype(ml_dtypes.bfloat16)
    in_maps = [
        {"x": flat[i * ROWS_PER_CORE:(i + 1) * ROWS_PER_CORE], "bd": bd}
        for i in range(N_CORES)
    ]
    nc = _get_nc()
    res = run_bass_kernel_spmd(nc, in_maps, core_ids=list(range(N_CORES)))
    out = np.empty((B * C * H, W), dtype=np.float32)
    for i in range(N_CORES):
        out[i * ROWS_PER_CORE:(i + 1) * ROWS_PER_CORE] = np.asarray(
            res.results[i]["out"], dtype=np.float32)
    return out.reshape(B, C, H, W)


# revision 7
# speedup vs baseline: 1.7835x; 1.0088x over previous
"""8x8 blockwise 2D DCT on x[16,32,512,512] f32, data-parallel on 8 TRN2 cores.

Math: per 8x8 block Blk of the image, coeffs = D @ Blk @ D^T.  With
BD = blockdiag_16(D^T) [128,128], a [128h x 128w] chunk X satisfies:

  mm1: P1 = X^T  @ BD   (contracts h: column-DCT, output lands as [w, h'])
  mm2: P2 = P1^T @ BD   (contracts w: row-DCT,    output lands as [h', w'])

Both matmuls use the data chunk as the stationary operand (lhsT) and BD as
the moving operand, so each pass both applies the DCT and transposes -- two
passes return to the original orientation with zero explicit transposes.

I/O is bf16 end to end: the host pre-casts x f32->bf16 (identical rounding
to an in-DMA cast) and upcasts the bf16 result back to f32.  This halves
HBM traffic per core to 32 MiB in + 32 MiB out over ~358 GB/s => ~195 us
DMA floor (vs ~375 us for f32 I/O).  Measured rel err ~3.3e-3 (gate 2e-2).

Sharding: pure data parallel along batch -- core i takes x[2i:2i+2],
viewed flat as [32768, 512] rows, processed as 32 macro-tiles of 8 slabs
([128, 512] bf16 each) with 1 MiB contiguous DMAs.

Engine schedule (the part that matters): PSUM evacuations from a strict-
FIFO engine must never wait on work that depends on an earlier entry of
the same FIFO.  So stage 1 (4x mm1 + one PAIR-WIDE [128,1024] evac on
DVE) and stage 2 (4x mm2 + pair-wide evac on ACT) are software-pipelined
with stage 2 deferred by PIPE_DEPTH slab-pairs: the PE always has an
independent mm1 group in flight while DVE drains ps1, DVE only ever runs
evac1s back to back, ACT only evac2s.  Pair-wide evacs amortize the
~120-172 cy per-instruction PSUM-read overhead (PSUM-source copies run
1 elem/cycle -- no packed mode from PSUM on TRN2).  Loads ride SWDGE
(head tiles also pull on the idle HWDGE ring during the fill window),
stores alternate both HWDGE rings and rotate 3-way at the drain.
"""

import numpy as np
import ml_dtypes

import concourse.bacc as bacc
import concourse.mybir as mybir
from concourse import tile
from concourse.bass_utils import run_bass_kernel_spmd

N_CORES = 8
B, C, H, W = 16, 32, 512, 512
ROWS_PER_CORE = (B // N_CORES) * C * H  # 32768
SLABS = ROWS_PER_CORE // 128            # 256
NSLAB = 8                               # slabs per macro-tile (1 MiB bf16 DMAs)
PAIRS_PER_TILE = NSLAB // 2

# Tuning knobs (env-overridable for A/B)
import os as _os
IN_BUFS = int(_os.environ.get("DCT_IN_BUFS", "10"))
OUT_BUFS = int(_os.environ.get("DCT_OUT_BUFS", "6"))
HEAD_DUAL = int(_os.environ.get("DCT_HEAD_DUAL", "8"))
TAIL_3WAY = int(_os.environ.get("DCT_TAIL_3WAY", "6"))
# software-pipeline depth in slab-pairs: stage 2 of pair q issues after
# stage 1 of pair q+PIPE_DEPTH (keeps PE fed while DVE evacuates ps1)
PIPE_DEPTH = int(_os.environ.get("DCT_PIPE_DEPTH", "1"))
MID_BUFS = int(_os.environ.get("DCT_MID_BUFS", "4"))
PS_BUFS = int(_os.environ.get("DCT_PS_BUFS", "2"))

_cached_nc = None


def _build_nc():
    f32 = mybir.dt.float32
    bf16 = mybir.dt.bfloat16
    nc = bacc.Bacc("TRN2", target_bir_lowering=False, debug=False,
                   num_devices=N_CORES)
    x_ext = nc.declare_dram_parameter("x", [ROWS_PER_CORE, W], bf16,
                                      isOutput=False)
    bd_ext = nc.declare_dram_parameter("bd", [128, 128], bf16, isOutput=False)
    out_ext = nc.declare_dram_parameter("out", [ROWS_PER_CORE, W], bf16,
                                        isOutput=True)

    with tile.TileContext(nc) as tc:
        with (
            tc.tile_pool(name="const", bufs=1) as cpool,
            tc.tile_pool(name="xin", bufs=IN_BUFS) as xpool,
            tc.tile_pool(name="mid", bufs=MID_BUFS) as mpool,
            tc.tile_pool(name="oout", bufs=OUT_BUFS) as opool,
            tc.tile_pool(name="ps1p", bufs=PS_BUFS, space="PSUM") as ps1pool,
            tc.tile_pool(name="ps2p", bufs=PS_BUFS, space="PSUM") as ps2pool,
        ):
            bd16 = cpool.tile([128, 128], bf16)
            nc.sync.dma_start(bd16[:], bd_ext[:, :])

            n_tiles = SLABS // NSLAB

            def store_tile(t, otp):
                r0 = t * NSLAB * 128
                dst = out_ext[r0:r0 + NSLAB * 128, :].rearrange(
                    "(n p) w -> p n w", p=128)
                if t >= n_tiles - TAIL_3WAY:
                    store_eng = [nc.sync, nc.scalar, nc.gpsimd][t % 3]
                else:
                    store_eng = nc.sync if t % 2 == 0 else nc.scalar
                store_eng.dma_start(dst,
                                    otp.rearrange("p (n w) -> p n w", n=NSLAB))

            def stage2(p):
                t1p, otp, q, tp = p
                ps2 = ps2pool.tile([128, 1024], f32, tag="ps2")
                for c in range(8):
                    nc.tensor.matmul(
                        ps2[:, c * 128:(c + 1) * 128],
                        lhsT=t1p[:, c * 128:(c + 1) * 128],
                        rhs=bd16[:],
                        start=True, stop=True)
                nc.scalar.copy(otp[:, q * 1024:(q + 1) * 1024], ps2[:])
                if q == PAIRS_PER_TILE - 1:
                    store_tile(tp, otp)

            pend = []  # stage-1-done slab-pairs awaiting stage 2
            for t in range(n_tiles):
                r0 = t * NSLAB * 128
                xt = xpool.tile([128, NSLAB * W], bf16, tag="xt")
                src = x_ext[r0:r0 + NSLAB * 128, :].rearrange(
                    "(n p) w -> p n w", p=128)
                xtv = xt.rearrange("p (n w) -> p n w", n=NSLAB)
                if t < HEAD_DUAL and t % 2 == 0:
                    nc.sync.dma_start(xtv, src)
                else:
                    nc.gpsimd.dma_start(xtv, src)

                ot = opool.tile([128, NSLAB * W], bf16, tag="ot")
                for q in range(PAIRS_PER_TILE):
                    ps1 = ps1pool.tile([128, 1024], f32, tag="ps1")
                    for c in range(8):
                        nc.tensor.matmul(
                            ps1[:, c * 128:(c + 1) * 128],
                            lhsT=xt[:, q * 1024 + c * 128:
                                    q * 1024 + (c + 1) * 128],
                            rhs=bd16[:],
                            start=True, stop=True)
                    t1 = mpool.tile([128, 1024], bf16, tag="t1")
                    nc.vector.tensor_copy(t1[:], ps1[:])
                    pend.append((t1, ot, q, t))
                    if len(pend) > PIPE_DEPTH:
                        stage2(pend.pop(0))
            for p in pend:
                stage2(p)
    nc.compile()
    return nc


def _get_nc():
    global _cached_nc
    if _cached_nc is None:
        _cached_nc = _build_nc()
    return _cached_nc


def kernel(x, dct_matrix):
    x = np.asarray(x, dtype=np.float32)
    d = np.asarray(dct_matrix, dtype=np.float32)
    assert x.shape == (B, C, H, W), x.shape
    assert d.shape == (8, 8), d.shape

    bd = np.kron(np.eye(16, dtype=np.float32),
                 d.T).astype(ml_dtypes.bfloat16)
    flat = x.reshape(B * C * H, W).astype(ml_dtypes.bfloat16)
    in_maps = [
        {"x": flat[i * ROWS_PER_CORE:(i + 1) * ROWS_PER_CORE], "bd": bd}
        for i in range(N_CORES)
    ]
    nc = _get_nc()
    res = run_bass_kernel_spmd(nc, in_maps, core_ids=list(range(N_CORES)))
    out = np.empty((B * C * H, W), dtype=np.float32)
    for i in range(N_CORES):
        out[i * ROWS_PER_CORE:(i + 1) * ROWS_PER_CORE] = np.asarray(
            res.results[i]["out"], dtype=np.float32)
    return out.reshape(B, C, H, W)
